# revision 15
# baseline (speedup 1.0000x reference)
"""Trainium2 Bass kernel for a custom LSTM cell step.

Reference computation (per full problem, B=8192, D=U=512):
    z = inputs @ kernel + h_tm1 @ recurrent_kernel + bias        # [B, 4U]
    i, f, g, o = split(z, 4, axis=1)
    i, f, o = sigmoid(...)  ;  g = tanh(g)
    c = f * c_tm1 + i * g
    h = o * tanh(c)
    return (h, h, c)

Sharding: data-parallel over the batch dim across 8 NeuronCores
(1024 rows per core); kernel/recurrent_kernel/bias replicated.

Per-core kernel structure:
  - W[512,2048] and R[512,2048] form one stacked [1024, 2048] contraction
    (k-tiles 0-3 = W fed by x, 4-7 = R fed by h).
  - matmuls use the PE's full-rate float32r path (fp32 storage, 11-bit
    mantissa, 1 cycle/row at N=512 vs 4 cyc/row for fp32; transposes 1.5
    vs 2). x, h, W, R are pre-rounded to the f32r subset on the host
    (RNE on the low 12 mantissa bits — bit-exact match to the device
    cast, and identical numerics to rounding after the transpose), so
    everything DMAs straight into SBUF with no staging casts. c_tm1 and
    all post-matmul math stay exact fp32.
  - activations are transposed on the PE (128x128 tensor.transpose).
  - weights stream in N-chunks (one gate = 512 columns at a time, all K):
    each arriving chunk unlocks a complete 8-step PSUM accumulation for
    every m-tile, so PSUM banks live ~2us and the PE never waits on the
    weight stream once transposes are done.
  - emission interleaves transposes and gate phases to match the DMA
    arrival order, keeping the PE dense from ~10us to the end:
      T0-3 | i(mt0-3) | T4-7 | g(0-3) | i(4-7) | f(0-3) | g(4-7) |
      o(0-3) | f(4-7) | o(4-7)
    with DMA order xh[0:4], Wi, xh[4:8], Wg, c, Wf, Wo.
"""

from contextlib import ExitStack

import numpy as np

import concourse.bass as bass
import concourse.mybir as mybir
import concourse.tile as tile
from concourse import bacc
from concourse.bass_utils import run_bass_kernel_spmd
from concourse.masks import make_identity

# Problem sizes (hardcoded per spec).
B, D, U = 8192, 512, 512
N_CORES = 8
MB = B // N_CORES  # 1024 batch rows per core
P = 128
MT = MB // P  # 8 m-tiles per core
KO = (D + U) // P  # 8 stacked contraction tiles (4 from W/x, 4 from R/h)
NG = 4 * U  # 2048 gate columns

F32 = mybir.dt.float32
F32R = mybir.dt.float32r  # fp32 storage; fast PE matmul path

SIG = mybir.ActivationFunctionType.Sigmoid
TANH = mybir.ActivationFunctionType.Tanh

_NC_CACHE: dict = {}


def _round_f32r(a: np.ndarray) -> np.ndarray:
    """Round fp32 to the f32r-representable subset: RNE on the low 12
    mantissa bits. Bit-exact match to the on-device DVE f32->f32r cast
    (verified on hardware)."""
    u = a.astype(np.float32).view(np.uint32).astype(np.uint64)
    r = (u + 0x7FF + ((u >> 12) & 1)) & ~np.uint64(0xFFF)
    return r.astype(np.uint32).view(np.float32).reshape(a.shape)


def _build_lstm_nc(with_bias: bool):
    """Build and compile the per-core Bass program."""
    nc = bacc.Bacc("TRN2", target_bir_lowering=False, debug=False)

    # x/h/W/R arrive host-pre-rounded to the f32r subset.
    x_d = nc.dram_tensor("inputs", [MB, D], F32R, kind="ExternalInput")
    h_d = nc.dram_tensor("h_tm1", [MB, U], F32R, kind="ExternalInput")
    c_d = nc.dram_tensor("c_tm1", [MB, U], F32, kind="ExternalInput")
    w_d = nc.dram_tensor("kernel", [D, NG], F32R, kind="ExternalInput")
    r_d = nc.dram_tensor("recurrent_kernel", [U, NG], F32R, kind="ExternalInput")
    b_d = None
    if with_bias:
        b_d = nc.dram_tensor("bias", [NG], F32, kind="ExternalInput")
    ho_d = nc.dram_tensor("h_out", [MB, U], F32, kind="ExternalOutput")
    co_d = nc.dram_tensor("c_out", [MB, U], F32, kind="ExternalOutput")

    # DRAM views tiled to [partition, m_tile, free]
    x_v = x_d.ap().rearrange("(mt p) d -> p mt d", p=P)
    h_v = h_d.ap().rearrange("(mt p) d -> p mt d", p=P)
    c_v = c_d.ap().rearrange("(mt p) d -> p mt d", p=P)
    ho_v = ho_d.ap().rearrange("(mt p) d -> p mt d", p=P)
    co_v = co_d.ap().rearrange("(mt p) d -> p mt d", p=P)
    w_v = w_d.ap().rearrange("(ko p) n -> p ko n", p=P)
    r_v = r_d.ap().rearrange("(ko p) n -> p ko n", p=P)

    with tile.TileContext(nc) as tc, ExitStack() as ctx:
        consts = ctx.enter_context(tc.tile_pool(name="consts", bufs=1))
        xin = ctx.enter_context(tc.tile_pool(name="xin", bufs=2))
        xhT_pool = ctx.enter_context(tc.tile_pool(name="xhT", bufs=MT))
        keep = ctx.enter_context(tc.tile_pool(name="keep", bufs=4))
        scratch = ctx.enter_context(tc.tile_pool(name="scratch", bufs=3))
        outp = ctx.enter_context(tc.tile_pool(name="outp", bufs=3))
        zpsum = ctx.enter_context(tc.tile_pool(name="zpsum", bufs=6, space="PSUM"))
        tpsum = ctx.enter_context(tc.tile_pool(name="tpsum", bufs=2, space="PSUM"))

        # gpsimd memset can't target an f32r tile — build in f32, round via DVE.
        identity_f32 = consts.tile([P, P], F32)
        make_identity(nc, identity_f32)
        identity = consts.tile([P, P], F32R)
        nc.vector.tensor_copy(identity[:], identity_f32[:])

        c_sb = consts.tile([P, MT, U], F32)
        wr = consts.tile([P, KO, NG], F32R)

        # --- DMA order on the sync ring (arrival-paced with emission) ---
        xs, hs = [], []

        def load_xh(g):
            sl = slice(2 * g, 2 * g + 2)
            xt = xin.tile([P, 2, D], F32R, tag="xstage")
            nc.sync.dma_start(xt[:], x_v[:, sl, :])
            ht = xin.tile([P, 2, U], F32R, tag="hstage")
            nc.sync.dma_start(ht[:], h_v[:, sl, :])
            xs.append(xt)
            hs.append(ht)

        def load_w_chunk(n):
            sl = slice(n * U, (n + 1) * U)
            nc.sync.dma_start(wr[:, 0:4, sl], w_v[:, :, sl])
            nc.sync.dma_start(wr[:, 4:8, sl], r_v[:, :, sl])

        load_xh(0)
        load_xh(1)
        load_w_chunk(0)  # i
        load_xh(2)
        load_xh(3)
        load_w_chunk(2)  # g
        nc.sync.dma_start(c_sb[:], c_v)
        load_w_chunk(1)  # f
        load_w_chunk(3)  # o

        bias_bc = None
        if with_bias:
            assert b_d is not None
            bias_bc = consts.tile([P, NG], F32)
            b_ap = b_d.ap()
            # DMA-replicate bias across all 128 partitions (partition step 0).
            nc.gpsimd.dma_start(
                out=bias_bc,
                in_=bass.AP(tensor=b_ap.tensor, offset=b_ap.offset, ap=[[0, P], [1, NG]]),
            )

        xhTs = {}

        def make_xhT(mt):
            """Transpose x/h m-tile [128, 512]x2 into a [k, m] tile [128, 8, 128]."""
            xhT = xhT_pool.tile([P, KO, P], F32R, tag="xhT")
            for gi, stage in enumerate((xs[mt // 2], hs[mt // 2])):
                tp = tpsum.tile([P, 4, P], F32R, tag="tp")  # one PSUM bank
                for k in range(4):
                    nc.tensor.transpose(
                        tp[:, k, :], stage[:, mt % 2, k * P : (k + 1) * P], identity
                    )
                nc.vector.tensor_copy(out=xhT[:, gi * 4 : (gi + 1) * 4, :], in_=tp[:])
            xhTs[mt] = xhT

        def z_chunk(n, mt):
            """Accumulate z[:, n*U:(n+1)*U] for m-tile mt into a PSUM bank."""
            zp = zpsum.tile([P, U], F32, tag="z")
            for ko in range(KO):
                nc.tensor.matmul(
                    zp[:],
                    xhTs[mt][:, ko, :],
                    wr[:, ko, n * U : (n + 1) * U],
                    start=(ko == 0),
                    stop=(ko == KO - 1),
                )
            if bias_bc is not None:
                nc.vector.tensor_add(zp[:], zp[:], bias_bc[:, n * U : (n + 1) * U])
            return zp

        i_t, ig_t, th_t = {}, {}, {}

        def phase_i(mt):  # i = sigmoid(z0)
            it = keep.tile([P, U], F32, tag="i")
            nc.scalar.activation(it[:], z_chunk(0, mt)[:], SIG)
            i_t[mt] = it

        def phase_g(mt):  # g = tanh(z2); ig = i*g
            gt = scratch.tile([P, U], F32, tag="gact")
            nc.scalar.activation(gt[:], z_chunk(2, mt)[:], TANH)
            ig = keep.tile([P, U], F32, tag="ig")
            nc.vector.tensor_mul(ig[:], i_t.pop(mt)[:], gt[:])
            ig_t[mt] = ig

        def phase_f(mt):  # f = sigmoid(z1); c = f*c_old + ig; tanh(c)
            ft = scratch.tile([P, U], F32, tag="gact")
            nc.scalar.activation(ft[:], z_chunk(1, mt)[:], SIG)
            c_new = outp.tile([P, U], F32, tag="cnew")
            nc.vector.tensor_mul(c_new[:], ft[:], c_sb[:, mt, :])
            nc.vector.tensor_add(c_new[:], c_new[:], ig_t.pop(mt)[:])
            nc.gpsimd.dma_start(co_v[:, mt, :], c_new[:])
            th = keep.tile([P, U], F32, tag="th")
            nc.scalar.activation(th[:], c_new[:], TANH)
            th_t[mt] = th

        def phase_o(mt):  # o = sigmoid(z3); h = o*tanh(c)
            ot = scratch.tile([P, U], F32, tag="gact")
            nc.scalar.activation(ot[:], z_chunk(3, mt)[:], SIG)
            h_new = outp.tile([P, U], F32, tag="hnew")
            nc.vector.tensor_mul(h_new[:], ot[:], th_t.pop(mt)[:])
            # h stores run late, when the sync ring is idle — HWDGE has the
            # lower completion latency, which shortens the kernel tail.
            nc.sync.dma_start(ho_v[:, mt, :], h_new[:])

        H0 = list(range(4))
        H1 = list(range(4, MT))

        # Emission order matched to DMA arrivals (PE stays dense from ~10us).
        for mt in H0:
            make_xhT(mt)
        for mt in H0:
            phase_i(mt)
        for mt in H1:
            make_xhT(mt)
        for mt in H0:
            phase_g(mt)
        for mt in H1:
            phase_i(mt)
        for mt in H0:
            phase_f(mt)
        for mt in H1:
            phase_g(mt)
        for mt in H0:
            phase_o(mt)
        for mt in H1:
            phase_f(mt)
        for mt in H1:
            phase_o(mt)

    nc.compile()
    return nc


def _get_nc(with_bias: bool):
    if with_bias not in _NC_CACHE:
        _NC_CACHE[with_bias] = _build_lstm_nc(with_bias)
    return _NC_CACHE[with_bias]


def kernel(inputs, h_tm1, c_tm1, kernel, recurrent_kernel, bias):
    x = _round_f32r(np.ascontiguousarray(np.asarray(inputs, dtype=np.float32)))
    h = _round_f32r(np.ascontiguousarray(np.asarray(h_tm1, dtype=np.float32)))
    c_tm1 = np.ascontiguousarray(np.asarray(c_tm1, dtype=np.float32))
    w = _round_f32r(np.ascontiguousarray(np.asarray(kernel, dtype=np.float32)))
    r = _round_f32r(np.ascontiguousarray(np.asarray(recurrent_kernel, dtype=np.float32)))
    b = np.ascontiguousarray(np.asarray(bias, dtype=np.float32))

    with_bias = bool(np.any(b))
    nc = _get_nc(with_bias)

    in_maps = []
    for core in range(N_CORES):
        sl = slice(core * MB, (core + 1) * MB)
        m = {
            "inputs": np.ascontiguousarray(x[sl]),
            "h_tm1": np.ascontiguousarray(h[sl]),
            "c_tm1": np.ascontiguousarray(c_tm1[sl]),
            "kernel": w,
            "recurrent_kernel": r,
        }
        if with_bias:
            m["bias"] = b
        in_maps.append(m)

    res = run_bass_kernel_spmd(nc, in_maps, core_ids=list(range(N_CORES)))
    h_out = np.concatenate([r_["h_out"] for r_ in res.results], axis=0)
    c_out = np.concatenate([r_["c_out"] for r_ in res.results], axis=0)
    return (h_out, h_out, c_out)


# revision 16
# speedup vs baseline: 1.0270x; 1.0270x over previous
"""Trainium2 Bass kernel for a custom LSTM cell step.

Reference computation (per full problem, B=8192, D=U=512):
    z = inputs @ kernel + h_tm1 @ recurrent_kernel + bias        # [B, 4U]
    i, f, g, o = split(z, 4, axis=1)
    i, f, o = sigmoid(...)  ;  g = tanh(g)
    c = f * c_tm1 + i * g
    h = o * tanh(c)
    return (h, h, c)

Sharding: data-parallel over the batch dim across 8 NeuronCores
(1024 rows per core); kernel/recurrent_kernel/bias replicated.

Per-core kernel structure:
  - W[512,2048] and R[512,2048] form one stacked [1024, 2048] contraction
    (k-tiles 0-3 = W fed by x, 4-7 = R fed by h).
  - matmuls use the PE's full-rate float32r path (fp32 storage, 11-bit
    mantissa, 1 cycle/row at N=512 vs 4 cyc/row for fp32; transposes 1.5
    vs 2). x, h, W, R are pre-rounded to the f32r subset on the host
    (RNE on the low 12 mantissa bits — bit-exact match to the device
    cast, and identical numerics to rounding after the transpose), so
    everything DMAs straight into SBUF with no staging casts. c_tm1 and
    all post-matmul math stay exact fp32.
  - activations are transposed on the PE (128x128 tensor.transpose).
  - weights stream in N-chunks (one gate = 512 columns at a time, all K):
    each arriving chunk unlocks a complete 8-step PSUM accumulation for
    every m-tile, so PSUM banks live ~2us and the PE never waits on the
    weight stream once transposes are done.
  - emission interleaves transposes and gate phases to match the DMA
    arrival order, keeping the PE dense from ~10us to the end:
      T0-3 | i(mt0-3) | T4-7 | g(0-3) | i(4-7) | f(0-3) | g(4-7) |
      o(0-3) | f(4-7) | o(4-7)
    with DMA order xh[0:4], Wi, xh[4:8], Wg, c, Wf, Wo.
"""

from contextlib import ExitStack

import numpy as np

import concourse.bass as bass
import concourse.mybir as mybir
import concourse.tile as tile
from concourse import bacc
from concourse.bass_utils import run_bass_kernel_spmd
from concourse.masks import make_identity

# Problem sizes (hardcoded per spec).
B, D, U = 8192, 512, 512
N_CORES = 8
MB = B // N_CORES  # 1024 batch rows per core
P = 128
MT = MB // P  # 8 m-tiles per core
KO = (D + U) // P  # 8 stacked contraction tiles (4 from W/x, 4 from R/h)
NG = 4 * U  # 2048 gate columns

F32 = mybir.dt.float32
F32R = mybir.dt.float32r  # fp32 storage; fast PE matmul path

SIG = mybir.ActivationFunctionType.Sigmoid
TANH = mybir.ActivationFunctionType.Tanh

_NC_CACHE: dict = {}


def _round_f32r(a: np.ndarray) -> np.ndarray:
    """Round fp32 to the f32r-representable subset: RNE on the low 12
    mantissa bits. Bit-exact match to the on-device DVE f32->f32r cast
    (verified on hardware)."""
    u = a.astype(np.float32).view(np.uint32).astype(np.uint64)
    r = (u + 0x7FF + ((u >> 12) & 1)) & ~np.uint64(0xFFF)
    return r.astype(np.uint32).view(np.float32).reshape(a.shape)


def _build_lstm_nc(with_bias: bool):
    """Build and compile the per-core Bass program."""
    nc = bacc.Bacc("TRN2", target_bir_lowering=False, debug=False)

    # x/h/W/R arrive host-pre-rounded to the f32r subset.
    x_d = nc.dram_tensor("inputs", [MB, D], F32R, kind="ExternalInput")
    h_d = nc.dram_tensor("h_tm1", [MB, U], F32R, kind="ExternalInput")
    c_d = nc.dram_tensor("c_tm1", [MB, U], F32, kind="ExternalInput")
    w_d = nc.dram_tensor("kernel", [D, NG], F32R, kind="ExternalInput")
    r_d = nc.dram_tensor("recurrent_kernel", [U, NG], F32R, kind="ExternalInput")
    b_d = None
    if with_bias:
        b_d = nc.dram_tensor("bias", [NG], F32, kind="ExternalInput")
    ho_d = nc.dram_tensor("h_out", [MB, U], F32, kind="ExternalOutput")
    co_d = nc.dram_tensor("c_out", [MB, U], F32, kind="ExternalOutput")

    # DRAM views tiled to [partition, m_tile, free]
    x_v = x_d.ap().rearrange("(mt p) d -> p mt d", p=P)
    h_v = h_d.ap().rearrange("(mt p) d -> p mt d", p=P)
    c_v = c_d.ap().rearrange("(mt p) d -> p mt d", p=P)
    ho_v = ho_d.ap().rearrange("(mt p) d -> p mt d", p=P)
    co_v = co_d.ap().rearrange("(mt p) d -> p mt d", p=P)
    w_v = w_d.ap().rearrange("(ko p) n -> p ko n", p=P)
    r_v = r_d.ap().rearrange("(ko p) n -> p ko n", p=P)

    with tile.TileContext(nc) as tc, ExitStack() as ctx:
        consts = ctx.enter_context(tc.tile_pool(name="consts", bufs=1))
        xin = ctx.enter_context(tc.tile_pool(name="xin", bufs=2))
        xhT_pool = ctx.enter_context(tc.tile_pool(name="xhT", bufs=MT))
        keep = ctx.enter_context(tc.tile_pool(name="keep", bufs=4))
        scratch = ctx.enter_context(tc.tile_pool(name="scratch", bufs=3))
        outp = ctx.enter_context(tc.tile_pool(name="outp", bufs=3))
        zpsum = ctx.enter_context(tc.tile_pool(name="zpsum", bufs=6, space="PSUM"))
        tpsum = ctx.enter_context(tc.tile_pool(name="tpsum", bufs=2, space="PSUM"))

        # gpsimd memset can't target an f32r tile — build in f32, round via DVE.
        identity_f32 = consts.tile([P, P], F32)
        make_identity(nc, identity_f32)
        identity = consts.tile([P, P], F32R)
        nc.vector.tensor_copy(identity[:], identity_f32[:])

        c_sb = consts.tile([P, MT, U], F32)
        wr = consts.tile([P, KO, NG], F32R)

        # --- DMA order on the sync ring (arrival-paced with emission) ---
        xs, hs = [], []

        def load_xh(g):
            sl = slice(2 * g, 2 * g + 2)
            xt = xin.tile([P, 2, D], F32R, tag="xstage")
            nc.sync.dma_start(xt[:], x_v[:, sl, :])
            ht = xin.tile([P, 2, U], F32R, tag="hstage")
            nc.sync.dma_start(ht[:], h_v[:, sl, :])
            xs.append(xt)
            hs.append(ht)

        def load_w_chunk(n):
            sl = slice(n * U, (n + 1) * U)
            nc.sync.dma_start(wr[:, 0:4, sl], w_v[:, :, sl])
            nc.sync.dma_start(wr[:, 4:8, sl], r_v[:, :, sl])

        load_xh(0)
        load_xh(1)
        load_w_chunk(0)  # i
        load_xh(2)
        load_xh(3)
        load_w_chunk(2)  # g
        nc.sync.dma_start(c_sb[:], c_v)
        load_w_chunk(1)  # f
        load_w_chunk(3)  # o

        bias_bc = None
        if with_bias:
            assert b_d is not None
            bias_bc = consts.tile([P, NG], F32)
            b_ap = b_d.ap()
            # DMA-replicate bias across all 128 partitions (partition step 0).
            nc.gpsimd.dma_start(
                out=bias_bc,
                in_=bass.AP(tensor=b_ap.tensor, offset=b_ap.offset, ap=[[0, P], [1, NG]]),
            )

        xhTs = {}

        def make_xhT(mt):
            """Transpose x/h m-tile [128, 512]x2 into a [k, m] tile [128, 8, 128]."""
            xhT = xhT_pool.tile([P, KO, P], F32R, tag="xhT")
            for gi, stage in enumerate((xs[mt // 2], hs[mt // 2])):
                tp = tpsum.tile([P, 4, P], F32R, tag="tp")  # one PSUM bank
                for k in range(4):
                    nc.tensor.transpose(
                        tp[:, k, :], stage[:, mt % 2, k * P : (k + 1) * P], identity
                    )
                nc.vector.tensor_copy(out=xhT[:, gi * 4 : (gi + 1) * 4, :], in_=tp[:])
            xhTs[mt] = xhT

        def z_chunk(n, mt):
            """Accumulate z[:, n*U:(n+1)*U] for m-tile mt into a PSUM bank."""
            zp = zpsum.tile([P, U], F32, tag="z")
            for ko in range(KO):
                nc.tensor.matmul(
                    zp[:],
                    xhTs[mt][:, ko, :],
                    wr[:, ko, n * U : (n + 1) * U],
                    start=(ko == 0),
                    stop=(ko == KO - 1),
                )
            if bias_bc is not None:
                nc.vector.tensor_add(zp[:], zp[:], bias_bc[:, n * U : (n + 1) * U])
            return zp

        i_t, ig_t, th_t = {}, {}, {}

        def phase_i(mt):  # i = sigmoid(z0)
            it = keep.tile([P, U], F32, tag="i")
            nc.scalar.activation(it[:], z_chunk(0, mt)[:], SIG)
            i_t[mt] = it

        def phase_g(mt):  # g = tanh(z2); ig = i*g
            gt = scratch.tile([P, U], F32, tag="gact")
            nc.scalar.activation(gt[:], z_chunk(2, mt)[:], TANH)
            ig = keep.tile([P, U], F32, tag="ig")
            nc.vector.tensor_mul(ig[:], i_t.pop(mt)[:], gt[:])
            ig_t[mt] = ig

        def phase_f(mt):  # f = sigmoid(z1); c = f*c_old + ig; tanh(c)
            ft = scratch.tile([P, U], F32, tag="gact")
            nc.scalar.activation(ft[:], z_chunk(1, mt)[:], SIG)
            c_new = outp.tile([P, U], F32, tag="cnew")
            nc.vector.tensor_mul(c_new[:], ft[:], c_sb[:, mt, :])
            nc.vector.tensor_add(c_new[:], c_new[:], ig_t.pop(mt)[:])
            nc.gpsimd.dma_start(co_v[:, mt, :], c_new[:])
            th = keep.tile([P, U], F32, tag="th")
            nc.scalar.activation(th[:], c_new[:], TANH)
            th_t[mt] = th

        def phase_o(mt):  # o = sigmoid(z3); h = o*tanh(c)
            ot = scratch.tile([P, U], F32, tag="gact")
            nc.scalar.activation(ot[:], z_chunk(3, mt)[:], SIG)
            h_new = outp.tile([P, U], F32, tag="hnew")
            nc.vector.tensor_mul(h_new[:], ot[:], th_t.pop(mt)[:])
            nc.gpsimd.dma_start(ho_v[:, mt, :], h_new[:])

        H0 = list(range(4))
        H1 = list(range(4, MT))

        # Emission order matched to DMA arrivals (PE stays dense from ~10us).
        for mt in H0:
            make_xhT(mt)
        for mt in H0:
            phase_i(mt)
        for mt in H1:
            make_xhT(mt)
        for mt in H0:
            phase_g(mt)
        for mt in H1:
            phase_i(mt)
        for mt in H0:
            phase_f(mt)
        for mt in H1:
            phase_g(mt)
        for mt in H0:
            phase_o(mt)
        for mt in H1:
            phase_f(mt)
        for mt in H1:
            phase_o(mt)

    nc.compile()
    return nc


def _get_nc(with_bias: bool):
    if with_bias not in _NC_CACHE:
        _NC_CACHE[with_bias] = _build_lstm_nc(with_bias)
    return _NC_CACHE[with_bias]


def kernel(inputs, h_tm1, c_tm1, kernel, recurrent_kernel, bias):
    x = _round_f32r(np.ascontiguousarray(np.asarray(inputs, dtype=np.float32)))
    h = _round_f32r(np.ascontiguousarray(np.asarray(h_tm1, dtype=np.float32)))
    c_tm1 = np.ascontiguousarray(np.asarray(c_tm1, dtype=np.float32))
    w = _round_f32r(np.ascontiguousarray(np.asarray(kernel, dtype=np.float32)))
    r = _round_f32r(np.ascontiguousarray(np.asarray(recurrent_kernel, dtype=np.float32)))
    b = np.ascontiguousarray(np.asarray(bias, dtype=np.float32))

    with_bias = bool(np.any(b))
    nc = _get_nc(with_bias)

    in_maps = []
    for core in range(N_CORES):
        sl = slice(core * MB, (core + 1) * MB)
        m = {
            "inputs": np.ascontiguousarray(x[sl]),
            "h_tm1": np.ascontiguousarray(h[sl]),
            "c_tm1": np.ascontiguousarray(c_tm1[sl]),
            "kernel": w,
            "recurrent_kernel": r,
        }
        if with_bias:
            m["bias"] = b
        in_maps.append(m)

    res = run_bass_kernel_spmd(nc, in_maps, core_ids=list(range(N_CORES)))
    h_out = np.concatenate([r_["h_out"] for r_ in res.results], axis=0)
    c_out = np.concatenate([r_["c_out"] for r_ in res.results], axis=0)
    return (h_out, h_out, c_out)
